# revision 5
# baseline (speedup 1.0000x reference)
"""Distributed multi-head self-attention for Trainium2 (8 NeuronCores).

Problem: b=4, n=2048, dim=1024, heads=16, dim_head=64.
  q = x@Wq; k,v = split(x@Wkv, 2); out = softmax(q k^T / 8) v; y = out@Wout + bout

Sharding: core c <-> (batch b=c//2, head-group g=c%2). Each core computes
q/k/v + attention for its batch's 8 heads (tensor-parallel columns of
Wq/Wkv). The pair (b,0)/(b,1) AllGathers the transposed bf16 attention
outputs (4 chunks of 2 heads each, overlapped with attention compute), then
each core runs the output projection with the full Wout over ITS HALF of the
sequence (selected from the gathered buffer with per-core one-hot mask
inputs, since the SPMD graph is identical on all cores). Core 2b+g emits out
rows [1024g : 1024(g+1)] of batch b; the host reassembles [4, 2048, 1024].

TensorEngine math is bf16 with f32 PSUM accumulation. Softmax skips
max-subtraction (scaled scores are ~N(0,1)); exp runs on the scalar engine
(PSUM f32 in -> bf16 SBUF out, scale fused). Denominators come from a ones
column appended to v; normalization = DVE reciprocal_approx_fast + GPSIMD
partition_broadcast + DVE multiply. Score matmuls (K=64) use tile_position
row groups. q/k projections for head-pair p+1 are emitted after attention(p)
so the TensorEngine fills its slack while attention is ACT(exp)-bound.
"""

import numpy as np

import concourse.mybir as mybir
import concourse.tile as tile
from concourse import bacc, bass_utils
from concourse.masks import make_identity

N_CORES = 8
B, N, D = 4, 2048, 1024
GH = 8          # heads per core
DH = 64
IN = GH * DH    # 512 inner dims per core
SCALE = DH ** -0.5
PT = 128
KD = D // PT    # 8 dim tiles
MS = N // PT    # 16 seq tiles
MI = IN // PT   # 4 head-pair tiles per core
NH = N // 2     # out rows per core
F32 = mybir.dt.float32
BF16 = mybir.dt.bfloat16

_COMPILED = None


def build():
    nc = bacc.Bacc("TRN2", target_bir_lowering=False, debug=False, num_devices=N_CORES)

    x_ext = nc.dram_tensor("x", [N, D], F32, kind="ExternalInput")
    wq_ext = nc.dram_tensor("wq", [D, IN], F32, kind="ExternalInput")
    wk_ext = nc.dram_tensor("wk", [D, IN], F32, kind="ExternalInput")
    wv_ext = nc.dram_tensor("wv", [D, IN], F32, kind="ExternalInput")
    wout_ext = nc.dram_tensor("wout", [D, D], F32, kind="ExternalInput")
    bout_ext = nc.dram_tensor("bout", [D], F32, kind="ExternalInput")
    sel_ext = nc.dram_tensor("sel", [1, 2], F32, kind="ExternalInput")
    out_ext = nc.dram_tensor("out", [NH, D], F32, kind="ExternalOutput")

    with tile.TileContext(nc) as tc:
        with (
            tc.tile_pool(name="const", bufs=1) as constp,
            tc.tile_pool(name="wpool", bufs=1) as wpool,
            tc.tile_pool(name="qkv", bufs=1) as qkv,
            tc.tile_pool(name="attout", bufs=1) as attoutp,
            tc.tile_pool(name="dram", bufs=1, space="DRAM") as dram,
        ):
            ident = constp.tile([PT, PT], BF16)
            make_identity(nc, ident[:])
            bias_row = constp.tile([1, D], F32)
            nc.sync.dma_start(bias_row[:], bout_ext[None, :])
            bias_bc = constp.tile([PT, D], F32)
            nc.gpsimd.partition_broadcast(bias_bc[:], bias_row[:])
            sel_row = constp.tile([1, 2], F32)
            nc.sync.dma_start(sel_row[:], sel_ext[:])
            s0_bc = constp.tile([PT, 1], F32)
            s1_bc = constp.tile([PT, 1], F32)
            nc.gpsimd.partition_broadcast(s0_bc[:], sel_row[:, 0:1])
            nc.gpsimd.partition_broadcast(s1_bc[:], sel_row[:, 1:2])

            wq_bf = [wpool.tile([PT, IN], BF16, name=f"wq_bf{k}") for k in range(KD)]
            wk_bf = [wpool.tile([PT, IN], BF16, name=f"wk_bf{k}") for k in range(KD)]
            wo_bf = [wpool.tile([PT, D], BF16, name=f"wo_bf{k}") for k in range(KD)]

            qT = [qkv.tile([PT, N], BF16, name=f"qT{m}") for m in range(MI)]
            kT = [qkv.tile([PT, N], BF16, name=f"kT{m}") for m in range(MI)]
            vsb = [qkv.tile([PT, GH, 66], BF16, name=f"v{s}") for s in range(MS)]

            attoutT = [attoutp.tile([PT, N], BF16, name=f"attoutT{p}") for p in range(MI)]
            attThalf = [attoutp.tile([PT, NH], BF16, name=f"attThalf{k}") for k in range(KD)]
            ag_in = [dram.tile([PT, N], BF16, name=f"ag_in{p}") for p in range(MI)]
            ag_out = [dram.tile([2 * PT, N], BF16, name=f"ag_out{p}") for p in range(MI)]

            # ================= phase 0: x first, then weights ==============
            with (
                tc.tile_pool(name="xT", bufs=1) as xTp,
                tc.tile_pool(name="stage", bufs=3) as stage,
                tc.tile_pool(name="xbf", bufs=2) as xbfp,
                tc.tile_pool(name="wvp", bufs=1) as wvp,
            ):
                xT = [xTp.tile([PT, N], BF16, name=f"xT{k}") for k in range(KD)]
                wv_bf = [wvp.tile([PT, IN], BF16, name=f"wv_bf{k}") for k in range(KD)]
                with tc.tile_pool(name="pst", bufs=6, space="PSUM") as pst:
                    for s in range(MS):
                        st = stage.tile([PT, D], F32, name="st", tag="st")
                        nc.sync.dma_start(st[:], x_ext[s * PT:(s + 1) * PT, :])
                        xbf = xbfp.tile([PT, D], BF16, name="xbf", tag="xbf")
                        nc.vector.tensor_copy(xbf[:], st[:])
                        for k in range(KD):
                            pt_ = pst.tile([PT, PT], BF16, name="pt_", tag="pt")
                            nc.tensor.transpose(
                                pt_[:], xbf[:, k * PT:(k + 1) * PT], ident[:]
                            )
                            nc.vector.tensor_copy(
                                xT[k][:, s * PT:(s + 1) * PT], pt_[:]
                            )
                    for k in range(KD):
                        for ext, dst in ((wv_ext, wv_bf), (wq_ext, wq_bf),
                                         (wk_ext, wk_bf)):
                            st = stage.tile([PT, D], F32, name="st", tag="st")
                            nc.sync.dma_start(st[:, :IN], ext[k * PT:(k + 1) * PT, :])
                            nc.vector.tensor_copy(dst[k][:], st[:, :IN])
                        st = stage.tile([PT, D], F32, name="st", tag="st")
                        nc.sync.dma_start(st[:], wout_ext[k * PT:(k + 1) * PT, :])
                        nc.vector.tensor_copy(wo_bf[k][:], st[:])

                # ============ phases 1+2 interleaved: proj + attention =====
                with (
                    tc.tile_pool(name="psP", bufs=1, space="PSUM") as psP,
                    tc.tile_pool(name="psS", bufs=2, space="PSUM") as psS,
                    tc.tile_pool(name="psO", bufs=3, space="PSUM") as psO,
                    tc.tile_pool(name="attn", bufs=4) as attnp,
                    tc.tile_pool(name="fin", bufs=2) as finp,
                    tc.tile_pool(name="agst", bufs=2) as agst,
                ):
                    def vproj():
                        for s in range(MS):
                            pv = psP.tile([PT, 512], F32, name="pv", tag="psP")
                            for k in range(KD):
                                nc.tensor.matmul(
                                    pv[:],
                                    xT[k][:, s * PT:(s + 1) * PT],
                                    wv_bf[k][:],
                                    start=(k == 0), stop=(k == KD - 1),
                                )
                            nc.gpsimd.memset(vsb[s][:, :, 64:65], 1.0)
                            nc.vector.tensor_copy(
                                vsb[s][:, :, 0:64],
                                pv[:].rearrange("p (h e) -> p h e", h=GH),
                            )

                    def qkproj(m):
                        for w_bf, dstT in ((wq_bf, qT), (wk_bf, kT)):
                            for ch in range(4):
                                ph = psP.tile([PT, 512], F32, name="ph", tag="psP")
                                for k in range(KD):
                                    nc.tensor.matmul(
                                        ph[:],
                                        w_bf[k][:, m * PT:(m + 1) * PT],
                                        xT[k][:, ch * 512:(ch + 1) * 512],
                                        start=(k == 0), stop=(k == KD - 1),
                                    )
                                nc.vector.tensor_copy(
                                    dstT[m][:, ch * 512:(ch + 1) * 512], ph[:]
                                )

                    def attention(p):
                        for iq in range(4):
                            oA = psO.tile([65, 512], F32, name="oA", tag="psO")
                            oB = psO.tile([65, 512], F32, name="oB", tag="psO")
                            for j in range(MS):
                                ps = psS.tile([PT, 1024], F32, name="ps", tag="psS")
                                nc.tensor.matmul(
                                    ps[:, 0:512],
                                    kT[p][0:64, j * PT:(j + 1) * PT],
                                    qT[p][0:64, iq * 512:(iq + 1) * 512],
                                    start=True, stop=True,
                                    tile_position=(0, 0),
                                )
                                nc.tensor.matmul(
                                    ps[:, 512:1024],
                                    kT[p][64:128, j * PT:(j + 1) * PT],
                                    qT[p][64:128, iq * 512:(iq + 1) * 512],
                                    start=True, stop=True,
                                    tile_position=(64, 0),
                                )
                                at = attnp.tile([PT, 1024], BF16, name="at", tag="at")
                                nc.scalar.activation(
                                    at[:], ps[:], mybir.ActivationFunctionType.Exp,
                                    scale=SCALE,
                                )
                                nc.tensor.matmul(
                                    oA[:], vsb[j][:, 2 * p, 0:65], at[:, 0:512],
                                    start=(j == 0), stop=(j == MS - 1),
                                )
                                nc.tensor.matmul(
                                    oB[:], vsb[j][:, 2 * p + 1, 0:65], at[:, 512:1024],
                                    start=(j == 0), stop=(j == MS - 1),
                                )
                            for hh, o in enumerate((oA, oB)):
                                recip = finp.tile([1, 512], F32, name="recip", tag="recip")
                                nc.vector.reciprocal(recip[:], o[64:65, :])
                                bc = finp.tile([64, 512], F32, name="bc", tag="bc")
                                nc.gpsimd.partition_broadcast(bc[:], recip[:])
                                nc.vector.tensor_tensor(
                                    attoutT[p][hh * 64:(hh + 1) * 64,
                                               iq * 512:(iq + 1) * 512],
                                    o[0:64, :], bc[:],
                                    op=mybir.AluOpType.mult,
                                )
                        nc.sync.dma_start(ag_in[p][:], attoutT[p][:])
                        nc.gpsimd.collective_compute(
                            "AllGather",
                            mybir.AluOpType.bypass,
                            replica_groups=[[0, 1], [2, 3], [4, 5], [6, 7]],
                            ins=[ag_in[p].opt()],
                            outs=[ag_out[p].opt()],
                        )
                        # stage + mask-select this pair's two gathered k-tiles
                        for kk in (p, p + MI):
                            half = kk // MI
                            ast = agst.tile([PT, N], BF16, name="ast", tag="ast")
                            nc.sync.dma_start(
                                ast[:], ag_out[p][half * PT:(half + 1) * PT, :]
                            )
                            tmp = agst.tile([PT, NH], BF16, name="tmp", tag="tmp")
                            nc.vector.tensor_scalar_mul(
                                tmp[:], ast[:, 0:NH], s0_bc[:]
                            )
                            nc.vector.scalar_tensor_tensor(
                                attThalf[kk][:],
                                ast[:, NH:N], s1_bc[:], tmp[:],
                                op0=mybir.AluOpType.mult,
                                op1=mybir.AluOpType.add,
                            )

                    vproj()
                    qkproj(0)
                    for p in range(MI):
                        attention(p)
                        if p + 1 < MI:
                            qkproj(p + 1)

                # ================= phase 3: output projection ==============
                with (
                    tc.tile_pool(name="pso", bufs=4, space="PSUM") as pso_p,
                    tc.tile_pool(name="osb", bufs=4) as osbp,
                ):
                    korder = [0, 4, 1, 5, 2, 6, 3, 7]
                    for m in range(NH // PT):
                        pso = [
                            pso_p.tile([PT, 512], F32, name="pso", tag="pso")
                            for _ in range(2)
                        ]
                        for ki, kk in enumerate(korder):
                            lhs = attThalf[kk][:, m * PT:(m + 1) * PT]
                            for nn in range(2):
                                nc.tensor.matmul(
                                    pso[nn][:],
                                    lhs,
                                    wo_bf[kk][:, nn * 512:(nn + 1) * 512],
                                    start=(ki == 0), stop=(ki == KD - 1),
                                )
                        for nn in range(2):
                            osb = osbp.tile([PT, 512], F32, name="osb", tag="osb")
                            nc.vector.tensor_tensor(
                                osb[:], pso[nn][:], bias_bc[:, nn * 512:(nn + 1) * 512],
                                op=mybir.AluOpType.add,
                            )
                            nc.sync.dma_start(
                                out_ext[m * PT:(m + 1) * PT, nn * 512:(nn + 1) * 512],
                                osb[:],
                            )

    nc.compile()
    return nc


def _shard_inputs(x, Wq, Wkv, Wout, bout):
    in_maps = []
    for c in range(N_CORES):
        b, g = c // 2, c % 2
        sel = np.zeros((1, 2), dtype=np.float32)
        sel[0, g] = 1.0
        in_maps.append({
            "x": np.ascontiguousarray(x[b], dtype=np.float32),
            "wq": np.ascontiguousarray(Wq[:, g * IN:(g + 1) * IN], dtype=np.float32),
            "wk": np.ascontiguousarray(Wkv[:, g * IN:(g + 1) * IN], dtype=np.float32),
            "wv": np.ascontiguousarray(
                Wkv[:, D + g * IN:D + (g + 1) * IN], dtype=np.float32
            ),
            "wout": np.ascontiguousarray(Wout, dtype=np.float32),
            "bout": np.ascontiguousarray(bout, dtype=np.float32),
            "sel": sel,
        })
    return in_maps


def kernel(x, Wq, Wkv, Wout, bout):
    global _COMPILED
    if _COMPILED is None:
        _COMPILED = build()
    nc = _COMPILED
    in_maps = _shard_inputs(
        np.asarray(x), np.asarray(Wq), np.asarray(Wkv), np.asarray(Wout),
        np.asarray(bout),
    )
    res = bass_utils.run_bass_kernel_spmd(nc, in_maps, core_ids=list(range(N_CORES)))
    out = np.empty((B, N, D), dtype=np.float32)
    for c in range(N_CORES):
        b, g = c // 2, c % 2
        out[b, g * NH:(g + 1) * NH, :] = res.results[c]["out"]
    return out


if __name__ == "__main__":
    rng = np.random.default_rng(0)
    x = rng.standard_normal((B, N, D)).astype(np.float32)
    Wq = rng.standard_normal((D, D)).astype(np.float32) * D ** -0.5
    Wkv = rng.standard_normal((D, 2 * D)).astype(np.float32) * D ** -0.5
    Wout = rng.standard_normal((D, D)).astype(np.float32) * D ** -0.5
    bout = np.zeros((D,), dtype=np.float32)
    y = kernel(x=x, Wq=Wq, Wkv=Wkv, Wout=Wout, bout=bout)
    print("out shape:", y.shape, "finite:", np.isfinite(y).all())


# revision 7
# speedup vs baseline: 1.1756x; 1.1756x over previous
"""Distributed multi-head self-attention for Trainium2 (8 NeuronCores).

Problem: b=4, n=2048, dim=1024, heads=16, dim_head=64.
  q = x@Wq; k,v = split(x@Wkv, 2); out = softmax(q k^T / 8) v; y = out@Wout + bout

Sharding: core c <-> (batch b=c//2, head-group g=c%2). Each core computes
q/k/v + attention for its batch's 8 heads (tensor-parallel columns of
Wq/Wkv). The pair (b,0)/(b,1) AllGathers the transposed bf16 attention
outputs (4 chunks of 2 heads each, overlapped with attention compute), then
each core runs the output projection with the full Wout over ITS HALF of the
sequence (selected from the gathered buffer with per-core one-hot mask
inputs, since the SPMD graph is identical on all cores). Core 2b+g emits out
rows [1024g : 1024(g+1)] of batch b; the host reassembles [4, 2048, 1024].

TensorEngine math is bf16 with f32 PSUM accumulation. Softmax skips
max-subtraction (scaled scores are ~N(0,1)); exp runs on the scalar engine
(PSUM f32 in -> bf16 SBUF out, scale fused). Denominators come from a ones
column appended to v; normalization = DVE reciprocal_approx_fast + GPSIMD
partition_broadcast + DVE multiply. Score matmuls (K=64) use tile_position
row groups. q/k projections for head-pair p+1 are emitted after attention(p)
so the TensorEngine fills its slack while attention is ACT(exp)-bound.
"""

import numpy as np

import concourse.mybir as mybir
import concourse.tile as tile
from concourse import bacc, bass_utils
from concourse.masks import make_identity

N_CORES = 8
B, N, D = 4, 2048, 1024
GH = 8          # heads per core
DH = 64
IN = GH * DH    # 512 inner dims per core
SCALE = DH ** -0.5
PT = 128
KD = D // PT    # 8 dim tiles
MS = N // PT    # 16 seq tiles
MI = IN // PT   # 4 head-pair tiles per core
NH = N // 2     # out rows per core
F32 = mybir.dt.float32
BF16 = mybir.dt.bfloat16

_COMPILED = None


def build():
    nc = bacc.Bacc("TRN2", target_bir_lowering=False, debug=False, num_devices=N_CORES)

    x_ext = nc.dram_tensor("x", [N, D], F32, kind="ExternalInput")
    wq_ext = nc.dram_tensor("wq", [D, IN], F32, kind="ExternalInput")
    wk_ext = nc.dram_tensor("wk", [D, IN], F32, kind="ExternalInput")
    wv_ext = nc.dram_tensor("wv", [D, IN], F32, kind="ExternalInput")
    wout_ext = nc.dram_tensor("wout", [D, D], F32, kind="ExternalInput")
    bout_ext = nc.dram_tensor("bout", [D], F32, kind="ExternalInput")
    sel_ext = nc.dram_tensor("sel", [1, 2], F32, kind="ExternalInput")
    out_ext = nc.dram_tensor("out", [NH, D], F32, kind="ExternalOutput")

    with tile.TileContext(nc) as tc:
        with (
            tc.tile_pool(name="const", bufs=1) as constp,
            tc.tile_pool(name="wpool", bufs=1) as wpool,
            tc.tile_pool(name="qkv", bufs=1) as qkv,
            tc.tile_pool(name="attout", bufs=1) as attoutp,
            tc.tile_pool(name="dram", bufs=1, space="DRAM") as dram,
        ):
            ident = constp.tile([PT, PT], BF16)
            make_identity(nc, ident[:])
            bias_row = constp.tile([1, D], F32)
            nc.sync.dma_start(bias_row[:], bout_ext[None, :])
            bias_bc = constp.tile([PT, D], F32)
            nc.gpsimd.partition_broadcast(bias_bc[:], bias_row[:])
            sel_row = constp.tile([1, 2], F32)
            nc.sync.dma_start(sel_row[:], sel_ext[:])
            s0_bc = constp.tile([PT, 1], F32)
            s1_bc = constp.tile([PT, 1], F32)
            nc.gpsimd.partition_broadcast(s0_bc[:], sel_row[:, 0:1])
            nc.gpsimd.partition_broadcast(s1_bc[:], sel_row[:, 1:2])

            wq_bf = [wpool.tile([PT, IN], BF16, name=f"wq_bf{k}") for k in range(KD)]
            wk_bf = [wpool.tile([PT, IN], BF16, name=f"wk_bf{k}") for k in range(KD)]
            wo_bf = [wpool.tile([PT, D], BF16, name=f"wo_bf{k}") for k in range(KD)]

            qT = [qkv.tile([PT, N], BF16, name=f"qT{m}") for m in range(MI)]
            kT = [qkv.tile([PT, N], BF16, name=f"kT{m}") for m in range(MI)]
            vsb = [qkv.tile([PT, GH, 66], BF16, name=f"v{s}") for s in range(MS)]

            attoutT = [attoutp.tile([PT, N], BF16, name=f"attoutT{p}") for p in range(MI)]
            attThalf = [attoutp.tile([PT, NH], BF16, name=f"attThalf{k}") for k in range(KD)]
            ag_in = [dram.tile([PT, N], BF16, name=f"ag_in{p}") for p in range(MI)]
            ag_out = [dram.tile([2 * PT, N], BF16, name=f"ag_out{p}") for p in range(MI)]

            # ================= phase 0: x first, then weights ==============
            with (
                tc.tile_pool(name="xT", bufs=1) as xTp,
                tc.tile_pool(name="stage", bufs=3) as stage,
                tc.tile_pool(name="xbf", bufs=2) as xbfp,
                tc.tile_pool(name="wvp", bufs=1) as wvp,
            ):
                xT = [xTp.tile([PT, N], BF16, name=f"xT{k}") for k in range(KD)]
                wv_bf = [wvp.tile([PT, IN], BF16, name=f"wv_bf{k}") for k in range(KD)]
                with tc.tile_pool(name="pst", bufs=6, space="PSUM") as pst:
                    for s in range(MS):
                        st = stage.tile([PT, D], F32, name="st", tag="st")
                        nc.sync.dma_start(st[:], x_ext[s * PT:(s + 1) * PT, :])
                        xbf = xbfp.tile([PT, D], BF16, name="xbf", tag="xbf")
                        nc.vector.tensor_copy(xbf[:], st[:])
                        for k in range(KD):
                            pt_ = pst.tile([PT, PT], BF16, name="pt_", tag="pt")
                            nc.tensor.transpose(
                                pt_[:], xbf[:, k * PT:(k + 1) * PT], ident[:]
                            )
                            nc.vector.tensor_copy(
                                xT[k][:, s * PT:(s + 1) * PT], pt_[:]
                            )
                    for k in range(KD):
                        for ext, dst in ((wv_ext, wv_bf), (wq_ext, wq_bf),
                                         (wk_ext, wk_bf)):
                            st = stage.tile([PT, D], F32, name="st", tag="st")
                            nc.sync.dma_start(st[:, :IN], ext[k * PT:(k + 1) * PT, :])
                            nc.vector.tensor_copy(dst[k][:], st[:, :IN])
                        st = stage.tile([PT, D], F32, name="st", tag="st")
                        nc.sync.dma_start(st[:], wout_ext[k * PT:(k + 1) * PT, :])
                        nc.vector.tensor_copy(wo_bf[k][:], st[:])

                # ============ phases 1+2 interleaved: proj + attention =====
                with (
                    tc.tile_pool(name="psP", bufs=1, space="PSUM") as psP,
                    tc.tile_pool(name="psS", bufs=2, space="PSUM") as psS,
                    tc.tile_pool(name="psO", bufs=3, space="PSUM") as psO,
                    tc.tile_pool(name="attn", bufs=3) as attnp,
                    tc.tile_pool(name="fin", bufs=2) as finp,
                    tc.tile_pool(name="agst", bufs=2) as agst,
                ):
                    def vproj():
                        for s in range(MS):
                            pv = psP.tile([PT, 512], F32, name="pv", tag="psP")
                            for k in range(KD):
                                nc.tensor.matmul(
                                    pv[:],
                                    xT[k][:, s * PT:(s + 1) * PT],
                                    wv_bf[k][:],
                                    start=(k == 0), stop=(k == KD - 1),
                                )
                            nc.gpsimd.memset(vsb[s][:, :, 64:65], 1.0)
                            nc.vector.tensor_copy(
                                vsb[s][:, :, 0:64],
                                pv[:].rearrange("p (h e) -> p h e", h=GH),
                            )

                    def qkproj(m):
                        for w_bf, dstT in ((wq_bf, qT), (wk_bf, kT)):
                            for ch in range(4):
                                ph = psP.tile([PT, 512], F32, name="ph", tag="psP")
                                for k in range(KD):
                                    nc.tensor.matmul(
                                        ph[:],
                                        w_bf[k][:, m * PT:(m + 1) * PT],
                                        xT[k][:, ch * 512:(ch + 1) * 512],
                                        start=(k == 0), stop=(k == KD - 1),
                                    )
                                nc.vector.tensor_copy(
                                    dstT[m][:, ch * 512:(ch + 1) * 512], ph[:]
                                )

                    def attention(p):
                        for iq in range(4):
                            oA = psO.tile([65, 512], F32, name="oA", tag="psO")
                            oB = psO.tile([65, 512], F32, name="oB", tag="psO")
                            for j in range(MS):
                                ps = psS.tile([PT, 1024], F32, name="ps", tag="psS")
                                nc.tensor.matmul(
                                    ps[:, 0:512],
                                    kT[p][0:64, j * PT:(j + 1) * PT],
                                    qT[p][0:64, iq * 512:(iq + 1) * 512],
                                    start=True, stop=True,
                                    tile_position=(0, 0),
                                )
                                nc.tensor.matmul(
                                    ps[:, 512:1024],
                                    kT[p][64:128, j * PT:(j + 1) * PT],
                                    qT[p][64:128, iq * 512:(iq + 1) * 512],
                                    start=True, stop=True,
                                    tile_position=(64, 0),
                                )
                                at = attnp.tile([PT, 1024], BF16, name="at", tag="at")
                                nc.scalar.activation(
                                    at[:], ps[:], mybir.ActivationFunctionType.Exp,
                                    scale=SCALE,
                                )
                                nc.tensor.matmul(
                                    oA[:], vsb[j][:, 2 * p, 0:65], at[:, 0:512],
                                    start=(j == 0), stop=(j == MS - 1),
                                )
                                nc.tensor.matmul(
                                    oB[:], vsb[j][:, 2 * p + 1, 0:65], at[:, 512:1024],
                                    start=(j == 0), stop=(j == MS - 1),
                                )
                            for hh, o in enumerate((oA, oB)):
                                den = finp.tile([1, 512], F32, name="den", tag="den")
                                nc.vector.tensor_copy(den[:], o[64:65, :])
                                recip = finp.tile([1, 512], F32, name="recip", tag="recip")
                                nc.vector.reciprocal_approx_fast(recip[:], den[:])
                                bc = finp.tile([64, 512], F32, name="bc", tag="bc")
                                nc.gpsimd.partition_broadcast(bc[:], recip[:])
                                nc.vector.tensor_tensor(
                                    attoutT[p][hh * 64:(hh + 1) * 64,
                                               iq * 512:(iq + 1) * 512],
                                    o[0:64, :], bc[:],
                                    op=mybir.AluOpType.mult,
                                )
                        nc.sync.dma_start(ag_in[p][:], attoutT[p][:])
                        nc.gpsimd.collective_compute(
                            "AllGather",
                            mybir.AluOpType.bypass,
                            replica_groups=[[0, 1], [2, 3], [4, 5], [6, 7]],
                            ins=[ag_in[p].opt()],
                            outs=[ag_out[p].opt()],
                        )
                        # stage + mask-select this pair's two gathered k-tiles
                        for kk in (p, p + MI):
                            half = kk // MI
                            ast = agst.tile([PT, N], BF16, name="ast", tag="ast")
                            nc.sync.dma_start(
                                ast[:], ag_out[p][half * PT:(half + 1) * PT, :]
                            )
                            tmp = agst.tile([PT, NH], BF16, name="tmp", tag="tmp")
                            nc.vector.tensor_scalar_mul(
                                tmp[:], ast[:, 0:NH], s0_bc[:]
                            )
                            nc.vector.scalar_tensor_tensor(
                                attThalf[kk][:],
                                ast[:, NH:N], s1_bc[:], tmp[:],
                                op0=mybir.AluOpType.mult,
                                op1=mybir.AluOpType.add,
                            )

                    vproj()
                    qkproj(0)
                    for p in range(MI):
                        attention(p)
                        if p + 1 < MI:
                            qkproj(p + 1)

                # ================= phase 3: output projection ==============
                with (
                    tc.tile_pool(name="pso", bufs=4, space="PSUM") as pso_p,
                    tc.tile_pool(name="osb", bufs=4) as osbp,
                ):
                    korder = [0, 4, 1, 5, 2, 6, 3, 7]
                    for m in range(NH // PT):
                        pso = [
                            pso_p.tile([PT, 512], F32, name="pso", tag="pso")
                            for _ in range(2)
                        ]
                        for ki, kk in enumerate(korder):
                            lhs = attThalf[kk][:, m * PT:(m + 1) * PT]
                            for nn in range(2):
                                nc.tensor.matmul(
                                    pso[nn][:],
                                    lhs,
                                    wo_bf[kk][:, nn * 512:(nn + 1) * 512],
                                    start=(ki == 0), stop=(ki == KD - 1),
                                )
                        for nn in range(2):
                            osb = osbp.tile([PT, 512], F32, name="osb", tag="osb")
                            nc.vector.tensor_tensor(
                                osb[:], pso[nn][:], bias_bc[:, nn * 512:(nn + 1) * 512],
                                op=mybir.AluOpType.add,
                            )
                            nc.sync.dma_start(
                                out_ext[m * PT:(m + 1) * PT, nn * 512:(nn + 1) * 512],
                                osb[:],
                            )

    nc.compile()
    return nc


def _shard_inputs(x, Wq, Wkv, Wout, bout):
    in_maps = []
    for c in range(N_CORES):
        b, g = c // 2, c % 2
        sel = np.zeros((1, 2), dtype=np.float32)
        sel[0, g] = 1.0
        in_maps.append({
            "x": np.ascontiguousarray(x[b], dtype=np.float32),
            "wq": np.ascontiguousarray(Wq[:, g * IN:(g + 1) * IN], dtype=np.float32),
            "wk": np.ascontiguousarray(Wkv[:, g * IN:(g + 1) * IN], dtype=np.float32),
            "wv": np.ascontiguousarray(
                Wkv[:, D + g * IN:D + (g + 1) * IN], dtype=np.float32
            ),
            "wout": np.ascontiguousarray(Wout, dtype=np.float32),
            "bout": np.ascontiguousarray(bout, dtype=np.float32),
            "sel": sel,
        })
    return in_maps


def kernel(x, Wq, Wkv, Wout, bout):
    global _COMPILED
    if _COMPILED is None:
        _COMPILED = build()
    nc = _COMPILED
    in_maps = _shard_inputs(
        np.asarray(x), np.asarray(Wq), np.asarray(Wkv), np.asarray(Wout),
        np.asarray(bout),
    )
    res = bass_utils.run_bass_kernel_spmd(nc, in_maps, core_ids=list(range(N_CORES)))
    out = np.empty((B, N, D), dtype=np.float32)
    for c in range(N_CORES):
        b, g = c // 2, c % 2
        out[b, g * NH:(g + 1) * NH, :] = res.results[c]["out"]
    return out


if __name__ == "__main__":
    rng = np.random.default_rng(0)
    x = rng.standard_normal((B, N, D)).astype(np.float32)
    Wq = rng.standard_normal((D, D)).astype(np.float32) * D ** -0.5
    Wkv = rng.standard_normal((D, 2 * D)).astype(np.float32) * D ** -0.5
    Wout = rng.standard_normal((D, D)).astype(np.float32) * D ** -0.5
    bout = np.zeros((D,), dtype=np.float32)
    y = kernel(x=x, Wq=Wq, Wkv=Wkv, Wout=Wout, bout=bout)
    print("out shape:", y.shape, "finite:", np.isfinite(y).all())
